# revision 1
# baseline (speedup 1.0000x reference)
"""Trainium2 Bass kernel for nn_CrossAttentionModule (cross-attention token
re-weighting): per batch, L2-normalize 196 tokens of class/query feats over
C=640 channels, corr = ct @ qt^T, tiny MLP on corr means -> kernel vector,
softmax(corr @ kernel / T) -> per-token attention, out = feat * (1 + attn).

Sharding: pure data parallel, B=512 -> 64 batches on each of 8 NeuronCores.
"""
import numpy as np

try:
    import concourse.bass as bass
except ImportError:  # fresh grading dir: toolchain lives in /opt/trn_rl_repo
    import sys
    sys.path.insert(0, "/opt/trn_rl_repo")
    import concourse.bass as bass

import bass_rust
import concourse.mybir as mybir
from concourse import tile
from concourse.bass_utils import run_bass_kernel_spmd
from concourse.vector_clock import ScopedClock

F32 = mybir.dt.float32
F32R = mybir.dt.float32r
AF = mybir.ActivationFunctionType
ALU = mybir.AluOpType

C = 640          # channels
T = 196          # tokens (14*14)
NCH = 5          # C / 128 chunks
TA, TB = 128, 68  # token chunks
INV_TEMP = 40.0  # 1 / 0.025
N_CORES = 8


def _patched_drain_and_barrier(self, tick_clock, wait_clock):
    # Walrus here rejects >2 sync waits on one instruction ("Too many sync
    # wait commands"). Emit one wait_ge per semaphore, then a bare drain.
    probe = self.nc.sync.nop()
    wait_clock.add_sem_waits(probe.ins, ScopedClock({None: tick_clock.global_clock}))
    si = probe.ins.sync_info
    waits = list(si.on_wait) if si is not None else []
    probe.ins.sync_info = bass_rust.SyncInfo(
        on_wait=[], on_update=list(si.on_update) if si is not None else []
    )
    handles = {h.name: h for h in self.sems.allocated().values()}
    for w in waits:
        self.nc.sync.wait_ge(handles[w.ant_name], w.wait_value)
    self.nc.sync.drain()
    self.nc.all_engine_barrier()
    popped = self.nc._tile_sem_poison_stack.pop()
    assert popped is self._sem_poison
    self.nc.clear_and_free_semaphores(list(self.sems.allocated().values()))
    self.nc.all_engine_barrier()


tile.TileContext._drain_and_barrier = _patched_drain_and_barrier

WAIT_LIMIT = 1  # max sem waits walrus accepts on one instruction


def _split_waits(nc, limit=None):
    if limit is None:
        limit = WAIT_LIMIT
    """Walrus rejects instructions carrying more than `limit` sync waits.
    Move excess waits onto same-engine NoOps inserted just before."""
    n_split = 0
    for fn in nc.m.functions:
        for blk in fn.blocks:
            il = blk.instructions  # live list
            idx = 0
            while idx < len(il):
                inst = il[idx]
                si = getattr(inst, "sync_info", None)
                if si is not None and len(si.on_wait) > limit:
                    waits = list(si.on_wait)
                    inst.sync_info = bass_rust.SyncInfo(
                        on_wait=waits[:limit], on_update=list(si.on_update))
                    extra = waits[limit:]
                    pos = idx
                    for j in range(0, len(extra), limit):
                        nop = mybir.InstNoOp(
                            name=f"wsplit-{nc.next_id()}", ins=[], outs=[])
                        nop.engine = inst.engine
                        nop.sync_info = bass_rust.SyncInfo(
                            on_wait=extra[j:j + limit], on_update=[])
                        il.insert(pos, nop)
                        pos += 1
                        idx += 1
                        n_split += 1
                idx += 1
    return n_split


def build_nc(bpc, trace_sim=False, reps=1, loop_reps=1):
    nc = bass.Bass("TRN2", target_bir_lowering=False, debug=False,
                   num_devices=N_CORES)
    cf = nc.dram_tensor("cf", [bpc, C, T], F32, kind="ExternalInput").ap()
    qf = nc.dram_tensor("qf", [bpc, C, T], F32, kind="ExternalInput").ap()
    # stacked MLP weights: index 0 block = query branch, 1 block = class
    w1s = nc.dram_tensor("w1s", [T, 64], F32, kind="ExternalInput").ap()
    w2s = nc.dram_tensor("w2s", [64, T], F32, kind="ExternalInput").ap()
    b1s = nc.dram_tensor("b1s", [64, 1], F32, kind="ExternalInput").ap()
    b2s = nc.dram_tensor("b2s", [T, 2], F32, kind="ExternalInput").ap()
    onec = nc.dram_tensor("onec", [128, 1], F32, kind="ExternalInput").ap()
    imc = nc.dram_tensor("imc", [128, 1], F32, kind="ExternalInput").ap()
    oner = nc.dram_tensor("oner", [1, 128], F32, kind="ExternalInput").ap()
    i128 = nc.dram_tensor("i128", [128, 128], F32, kind="ExternalInput").ap()
    oner392 = nc.dram_tensor("oner392", [1, 392], F32, kind="ExternalInput").ap()
    co = nc.dram_tensor("co", [bpc, C, T], F32, kind="ExternalOutput").ap()
    qo = nc.dram_tensor("qo", [bpc, C, T], F32, kind="ExternalOutput").ap()

    with tile.TileContext(nc, trace_sim=trace_sim) as tc:
        with (
            tc.tile_pool(name="const", bufs=1) as cp,
            tc.tile_pool(name="xp", bufs=3) as xp,
            tc.tile_pool(name="xrp", bufs=3) as xrp,
            tc.tile_pool(name="sqp", bufs=3) as sqp,
            tc.tile_pool(name="op", bufs=4) as op,
            tc.tile_pool(name="pqp", bufs=3) as pqp,
            tc.tile_pool(name="smp", bufs=4) as smp,
            tc.tile_pool(name="gps", bufs=1, space="PSUM") as gps,
            tc.tile_pool(name="rowps", bufs=1, space="PSUM") as rowps,
            tc.tile_pool(name="smallps", bufs=2, space="PSUM") as smallps,
            tc.tile_pool(name="rbcps", bufs=1, space="PSUM") as rbcps,
        ):
            # ---- persistent constants ----
            w1a = cp.tile([TA, 64], F32)
            w1b = cp.tile([TB, 64], F32)
            w2_sb = cp.tile([64, T], F32)
            b1_sb = cp.tile([64, 1], F32)
            b2a = cp.tile([TA, 2], F32)
            b2b = cp.tile([TB, 2], F32)
            onec_sb = cp.tile([128, 1], F32)
            imc_sb = cp.tile([128, 1], F32)
            oner_sb = cp.tile([1, 128], F32)
            id_sb = cp.tile([128, 128], F32)
            nc.sync.dma_start(w1a[:, :], w1s[0:TA, :])
            nc.sync.dma_start(w1b[:, :], w1s[TA:T, :])
            nc.sync.dma_start(w2_sb[:, :], w2s[:, :])
            nc.sync.dma_start(b1_sb[:, :], b1s[:, :])
            nc.sync.dma_start(b2a[:, :], b2s[0:TA, :])
            nc.sync.dma_start(b2b[:, :], b2s[TA:T, :])
            nc.sync.dma_start(onec_sb[:, :], onec[:, :])
            nc.sync.dma_start(imc_sb[:, :], imc[:, :])
            nc.sync.dma_start(oner_sb[:, :], oner[:, :])
            nc.sync.dma_start(id_sb[:, :], i128[:, :])
            # fp32r-rounded constants (fp32r matmul operands must come from a
            # rounding producer instruction, not straight from DMA)
            onec_r = cp.tile([128, 1], F32R)
            oner_r = cp.tile([1, 128], F32R)
            id_r = cp.tile([128, 128], F32R)
            oner392_sb = cp.tile([1, 392], F32)
            nc.sync.dma_start(oner392_sb[:, :], oner392[:, :])
            ones392_r = cp.tile([1, 392], F32R)
            nc.vector.tensor_copy(ones392_r[:, :], oner392_sb[:, :])
            nc.vector.tensor_copy(onec_r[:, :], onec_sb[:, :])
            nc.vector.tensor_copy(oner_r[:, :], oner_sb[:, :])
            nc.vector.tensor_copy(id_r[:, :], id_sb[:, :])

            batches = [bb for _ in range(reps) for bb in range(bpc)]

            def load_x2(b0, nb):
                # x2[:, j, n, 0:196] = cls chunk n of batch b0+j, 196:392 qry
                x2 = xp.tile([128, 2, NCH, 2 * T], F32, name="x2")
                nc.sync.dma_start(
                    x2[:, 0:nb, :, 0:T],
                    cf[b0:b0 + nb].rearrange("b (n p) t -> p b n t", p=128))
                nc.sync.dma_start(
                    x2[:, 0:nb, :, T:2 * T],
                    qf[b0:b0 + nb].rearrange("b (n p) t -> p b n t", p=128))
                return x2

            def emit_batch(bi, b, x):
                # squares (GPSIMD; SBUF only)
                sq = sqp.tile([128, NCH, 2 * T], F32R, name="sq")
                nc.gpsimd.tensor_mul(sq[:, :, :], x[:, :, :], x[:, :, :])

                # ssq[t] = sum_c sq[c, t] for cls|qry -> [1, 392]
                ssq_ps = rowps.tile([1, 2 * T], F32, space="PSUM", tag="ssq",
                                    name="ssq_ps")
                for n in range(NCH):
                    nc.tensor.matmul(ssq_ps[:, :], onec_r[:, :], sq[:, n, :],
                                     start=(n == 0), stop=(n == NCH - 1))

                # ssq to per-partition columns (sc cols 0:4), rsqrt on DVE
                ssq_row = smp.tile([1, 2 * T], F32, name="ssq_row")
                nc.vector.tensor_copy(ssq_row[:, :], ssq_ps[:, :])
                sc_ps = smallps.tile([128, 16], F32, space="PSUM",
                                     tag="smallps", name="sc_ps")
                one1 = onec_sb[0:1, 0:1]
                nc.tensor.matmul(sc_ps[:, 0:1], ssq_row[:, 0:TA], one1,
                                 start=True, stop=True)
                nc.tensor.matmul(sc_ps[:, 1:2], ssq_row[:, T:T + TA], one1,
                                 start=True, stop=True)
                nc.tensor.matmul(sc_ps[0:TB, 2:3], ssq_row[:, TA:T], one1,
                                 start=True, stop=True)
                nc.tensor.matmul(sc_ps[0:TB, 3:4], ssq_row[:, T + TA:2 * T],
                                 one1, start=True, stop=True)
                # quake rsqrt + 2 Newton steps
                I32 = mybir.dt.int32
                sh = smp.tile([128, 4], I32, name="sh")
                nc.vector.tensor_scalar(sh[:, :], sc_ps[:, 0:4].bitcast(I32),
                                        1, None, ALU.logical_shift_right)
                y0i = smp.tile([128, 4], I32, name="y0i")
                nc.vector.tensor_scalar(y0i[:, :], sh[:, :], -1, 0x5F3759DF,
                                        ALU.mult, ALU.add)
                y = y0i[:, :].bitcast(F32)
                for it in range(2):
                    a2 = smp.tile([128, 4], F32, tag="nwa", bufs=4, name="a2")
                    nc.vector.tensor_mul(a2[:, :], y, y)
                    bsy = smp.tile([128, 4], F32, tag="nwb", bufs=4, name="bsy")
                    nc.vector.tensor_mul(bsy[:, :], a2[:, :], sc_ps[:, 0:4])
                    cny = smp.tile([128, 4], F32, tag="nwc", bufs=4, name="cny")
                    nc.vector.tensor_scalar(cny[:, :], bsy[:, :], -0.5, 1.5,
                                            ALU.mult, ALU.add)
                    yn = smp.tile([128, 4], F32, tag="nwy", bufs=4, name="yn")
                    nc.vector.tensor_mul(yn[:, :], y, cny[:, :])
                    y = yn[:, :]
                rcq = yn  # [128, 4]: 0:2 = (rc|rq) t/u 0:128, 2:4 = 128:196

                # raw gram G = cls^T @ x, fp32r full-width
                ga_ps = gps.tile([TA, 2 * T], F32, space="PSUM", tag="ga",
                                 name="ga_ps")
                gb_ps = gps.tile([TB, 2 * T], F32, space="PSUM", tag="gb",
                                 name="gb_ps")
                for out_ap, lsl_g in ((ga_ps[:, T:2 * T], slice(0, TA)),
                                      (gb_ps[:, T:2 * T], slice(TA, T))):
                    for n in range(NCH):
                        nc.tensor.matmul(out_ap, x[:, n, lsl_g],
                                         x[:, n, T:2 * T],
                                         start=(n == 0), stop=(n == NCH - 1))

                # P = diag(rc) G (DVE scaled copies, fp32r out)
                p_a = pqp.tile([TA, T], F32R, name="p_a")
                p_b = pqp.tile([TB, T], F32R, name="p_b")
                nc.vector.tensor_scalar_mul(p_a[:, :], ga_ps[:, T:2 * T],
                                            rcq[:, 0:1])
                nc.vector.tensor_scalar_mul(p_b[:, :], gb_ps[:, T:2 * T],
                                            rcq[0:TB, 2:3])

                # Q'[u, t] = P[t, u]^T (PE transpose, one PSUM bank)
                qp_ps = rowps.tile([128, 392], F32R, space="PSUM", tag="qp",
                                   name="qp_ps")
                nc.tensor.transpose(qp_ps[0:TA, 0:TA], p_a[:, 0:TA], id_r[:, :])
                nc.tensor.transpose(qp_ps[0:TA, TA:T], p_b[:, 0:TA],
                                    id_r[0:TB, 0:TB])
                nc.tensor.transpose(qp_ps[0:TB, T:T + TA], p_a[:, TA:T],
                                    id_r[:, :])
                nc.tensor.transpose(qp_ps[0:TB, T + TA:2 * T], p_b[:, TA:T],
                                    id_r[0:TB, 0:TB])
                q_a = pqp.tile([TA, T], F32R, name="q_a")
                q_b = pqp.tile([TB, T], F32R, name="q_b")
                nc.vector.tensor_copy(q_a[:, :], qp_ps[0:TA, 0:T])
                nc.vector.tensor_copy(q_b[:, :], qp_ps[0:TB, T:2 * T])

                # means (columns) into sc cols 4:8
                rq_col = (rcq[:, 1:2], rcq[0:TB, 3:4])
                ones_col = (onec_sb[0:TA, 0:1], onec_sb[0:TB, 0:1])
                for (osl_p, osl_c, lh_a, lh_b, rh) in (
                        (slice(0, TA), slice(4, 5), q_a, q_b, rq_col),
                        (slice(0, TA), slice(5, 6), p_a, p_b, ones_col),
                        (slice(0, TB), slice(6, 7), q_a, q_b, rq_col),
                        (slice(0, TB), slice(7, 8), p_a, p_b, ones_col)):
                    msl = slice(0, TA) if osl_c.start < 6 else slice(TA, T)
                    nc.tensor.matmul(sc_ps[osl_p, osl_c],
                                     lh_a[:, msl].bitcast(F32), rh[0],
                                     start=True, stop=False)
                    nc.tensor.matmul(sc_ps[osl_p, osl_c],
                                     lh_b[:, msl].bitcast(F32), rh[1],
                                     start=False, stop=True)
                mcol_a = smp.tile([TA, 2], F32, name="mcol_a")
                mcol_b = smp.tile([TB, 2], F32, name="mcol_b")
                nc.vector.tensor_copy(mcol_a[:, 0:1], sc_ps[0:TA, 4:5])
                nc.vector.tensor_mul(mcol_a[:, 1:2], sc_ps[0:TA, 5:6],
                                     rcq[:, 1:2])
                nc.vector.tensor_copy(mcol_b[:, 0:1], sc_ps[0:TB, 6:7])
                nc.vector.tensor_mul(mcol_b[:, 1:2], sc_ps[0:TB, 7:8],
                                     rcq[0:TB, 3:4])

                # MLP layer 1 -> sc cols 8:10
                nc.tensor.matmul(sc_ps[0:64, 8:10], w1a[:, :], mcol_a[:, :],
                                 start=True, stop=False)
                nc.tensor.matmul(sc_ps[0:64, 8:10], w1b[:, :], mcol_b[:, :],
                                 start=False, stop=True)
                z = smp.tile([64, 2], F32, name="z")
                nc.gpsimd.memset(z[:, :], 0.0)
                nc.vector.tensor_scalar(z[0:32, 0:1], sc_ps[0:32, 8:9],
                                        b1_sb[0:32, :], 0.0, ALU.add, ALU.max)
                nc.vector.tensor_scalar(z[32:64, 1:2], sc_ps[32:64, 9:10],
                                        b1_sb[32:64, :], 0.0, ALU.add, ALU.max)

                # MLP layer 2 -> sc cols 10:14
                nc.tensor.matmul(sc_ps[0:TA, 10:12], w2_sb[:, 0:TA], z[:, :],
                                 start=True, stop=True)
                nc.tensor.matmul(sc_ps[0:TB, 12:14], w2_sb[:, TA:T], z[:, :],
                                 start=True, stop=True)
                vq_a = smp.tile([TA, 1], F32, name="vq_a")
                vq_b = smp.tile([TB, 1], F32, name="vq_b")
                vc_a = smp.tile([TA, 1], F32R, name="vc_a")
                vc_b = smp.tile([TB, 1], F32R, name="vc_b")
                nc.vector.tensor_scalar(vq_a[:, :], sc_ps[0:TA, 10:11],
                                        b2a[:, 0:1], None, ALU.add)
                nc.vector.tensor_scalar(vq_b[:, :], sc_ps[0:TB, 12:13],
                                        b2b[:, 0:1], None, ALU.add)
                nc.vector.tensor_scalar(vc_a[:, :], sc_ps[0:TA, 11:12],
                                        b2a[:, 1:2], rcq[:, 1:2],
                                        ALU.add, ALU.mult)
                nc.vector.tensor_scalar(vc_b[:, :], sc_ps[0:TB, 13:14],
                                        b2b[:, 1:2], rcq[0:TB, 3:4],
                                        ALU.add, ALU.mult)

                # logits: c-branch row via Q'; q-branch columns via P
                lraw_ps = rowps.tile([1, 2 * T], F32, space="PSUM", tag="lraw",
                                     name="lraw_ps")
                nc.tensor.matmul(lraw_ps[:, T:2 * T], vc_a[:, :], q_a[:, :],
                                 start=True, stop=False)
                nc.tensor.matmul(lraw_ps[:, T:2 * T], vc_b[:, :], q_b[:, :],
                                 start=False, stop=True)
                nc.tensor.matmul(sc_ps[0:TA, 14:15], p_a[:, 0:TA].bitcast(F32),
                                 vq_a[:, :], start=True, stop=False)
                nc.tensor.matmul(sc_ps[0:TA, 14:15], p_b[:, 0:TA].bitcast(F32),
                                 vq_b[:, :], start=False, stop=True)
                nc.tensor.matmul(sc_ps[0:TB, 15:16], p_a[:, TA:T].bitcast(F32),
                                 vq_a[:, :], start=True, stop=False)
                nc.tensor.matmul(sc_ps[0:TB, 15:16], p_b[:, TA:T].bitcast(F32),
                                 vq_b[:, :], start=False, stop=True)
                lqs_a = smp.tile([TA, 1], F32, name="lqs_a")
                lqs_b = smp.tile([TB, 1], F32, name="lqs_b")
                nc.vector.tensor_mul(lqs_a[:, :], sc_ps[0:TA, 14:15],
                                     rcq[:, 1:2])
                nc.vector.tensor_mul(lqs_b[:, :], sc_ps[0:TB, 15:16],
                                     rcq[0:TB, 3:4])
                nc.tensor.matmul(lraw_ps[:, 0:TA], lqs_a[:, :], id_sb[:, :],
                                 start=True, stop=True)
                nc.tensor.matmul(lraw_ps[:, TA:T], lqs_b[:, :],
                                 id_sb[0:TB, 0:TB], start=True, stop=True)

                # softmax rows; ACT only does Exp (warm table)
                es = []
                for br in range(2):  # 0 = q-branch, 1 = c-branch
                    lg = lraw_ps[:, br * T:br * T + T]
                    nmx = smp.tile([1, 1], F32, tag="mx", bufs=4, name="nmx")
                    nc.vector.reduce_max(nmx[:, :], lg,
                                         axis=mybir.AxisListType.X, negate=True)
                    e = smp.tile([1, T], F32R, tag="e", bufs=4, name="e")
                    sm = smp.tile([1, 1], F32, tag="sm", bufs=4, name="sm")
                    nc.scalar.activation(e[:, :], lg, AF.Exp,
                                         bias=nmx[:, :], scale=1.0,
                                         accum_out=sm[:, :])
                    rs = smp.tile([1, 1], F32, tag="rs", bufs=4, name="rs")
                    nc.vector.reciprocal(rs[:, :], sm[:, :])
                    rsr = smp.tile([1, 128], F32R, tag="rsr", bufs=4,
                                   name="rsr")
                    nc.vector.tensor_scalar_mul(rsr[:, :], oner_sb[:, :],
                                                rs[:, :])
                    es.append((e, rsr))

                # bc = 1 + attn broadcast, accumulated in PSUM
                bc_ps = rbcps.tile([128, 2 * T], F32, space="PSUM", tag="rbc",
                                   name="bc_ps")
                nc.tensor.matmul(bc_ps[:, :], oner_r[:, :], ones392_r[:, :],
                                 start=True, stop=False, skip_group_check=True)
                nc.tensor.matmul(bc_ps[:, 0:T], es[1][1][:, :], es[1][0][:, :],
                                 start=False, stop=False, skip_group_check=True)
                nc.tensor.matmul(bc_ps[:, T:2 * T], es[0][1][:, :],
                                 es[0][0][:, :],
                                 start=False, stop=True, skip_group_check=True)
                o = op.tile([128, NCH, 2 * T], F32, name="o")
                for n in range(NCH):
                    nc.vector.tensor_mul(o[:, n, :], x[:, n, :], bc_ps[:, :])
                nc.scalar.dma_start(
                    co[b].rearrange("(n p) t -> p n t", p=128), o[:, :, 0:T])
                nc.scalar.dma_start(
                    qo[b].rearrange("(n p) t -> p n t", p=128), o[:, :, T:2 * T])

            def emit_all():
                pending = load_x2(batches[0], min(2, len(batches)))
                x2 = pending
                for bi, b in enumerate(batches):
                    if bi % 2 == 0:
                        x2 = pending
                        if bi + 2 < len(batches):
                            pending = load_x2(
                                batches[bi + 2], min(2, len(batches) - bi - 2))
                    emit_batch(bi, b, x2[:, bi % 2])

            if loop_reps > 1:
                with tc.For_i(0, loop_reps, 1):
                    emit_all()
            else:
                emit_all()
    _split_waits(nc)
    return nc


def _consts():
    return {
        "onec": np.ones((128, 1), np.float32),
        "imc": np.full((128, 1), 1.0 / T, np.float32),
        "oner": np.ones((1, 128), np.float32),
        "i128": np.eye(128, dtype=np.float32),
        "oner392": np.ones((1, 392), np.float32),
    }


_CACHE = {}


def prep_in_maps(class_feat, query_feat, cw1, cb1, cw2, cb2, qw1, qb1, qw2, qb2):
    B = class_feat.shape[0]
    bpc = B // N_CORES
    cfull = np.ascontiguousarray(np.asarray(class_feat, np.float32).reshape(B, C, T))
    qfull = np.ascontiguousarray(np.asarray(query_feat, np.float32).reshape(B, C, T))
    w1s = (np.concatenate([np.asarray(qw1), np.asarray(cw1)], axis=1)
           / T).astype(np.float32)
    w2s = (np.concatenate([np.asarray(qw2), np.asarray(cw2)], axis=0)
           * INV_TEMP).astype(np.float32)
    b1s = np.concatenate([np.asarray(qb1), np.asarray(cb1)])[:, None].astype(np.float32)
    b2s = (np.stack([np.asarray(qb2), np.asarray(cb2)], axis=1)
           * INV_TEMP).astype(np.float32)
    consts = _consts()
    in_maps = []
    for c in range(N_CORES):
        sl = slice(c * bpc, (c + 1) * bpc)
        in_maps.append({
            "cf": cfull[sl], "qf": qfull[sl],
            "w1s": w1s, "w2s": w2s, "b1s": b1s, "b2s": b2s, **consts,
        })
    return in_maps


def kernel(class_feat, query_feat, cw1, cb1, cw2, cb2, qw1, qb1, qw2, qb2):
    B = class_feat.shape[0]
    bpc = B // N_CORES
    if bpc not in _CACHE:
        _CACHE[bpc] = build_nc(bpc)
    nc = _CACHE[bpc]
    in_maps = prep_in_maps(class_feat, query_feat, cw1, cb1, cw2, cb2,
                           qw1, qb1, qw2, qb2)
    res = run_bass_kernel_spmd(nc, in_maps, core_ids=list(range(N_CORES)))
    S = int(np.sqrt(T))
    co = np.concatenate([res.results[c]["co"] for c in range(N_CORES)], axis=0)
    qo = np.concatenate([res.results[c]["qo"] for c in range(N_CORES)], axis=0)
    return (co.reshape(B, C, S, S), qo.reshape(B, C, S, S))

